# revision 14
# baseline (speedup 1.0000x reference)
"""Causal self-attention (B=16, T=1024, C=768, H=12) on 8 NeuronCores.

Strategy: data-parallel over batch (2 batches per core, no collectives).
All matmul operands bf16 (host-cast weights/x), f32 PSUM accumulation.

Per-core structure (2 batches x 6 head-pairs = 12 attention "pair slots"):
  - x^T tiles arrive via per-k-tile DMA-transposes (xbar) directly from
    HBM, interleaved with the W_qkv tile loads on the SP queue so the
    first projection matmul unblocks ~3us in; batch-1 transposes and
    W_proj ride the Activation DMA queue in parallel.
  - QKV / out-proj matmuls are split into small "filler jobs" (one PSUM
    bank each) woven between the attention phases of each pair so the
    PE stays dense while ScalarE runs the exp stream.
  - Attention per pair: S^T = K Q^T for both heads concurrently in the
    PE array (row groups 0-1 / 2-3, contraction 64 each), exp on
    ScalarE (scale=1/8) -> bf16, diagonal-block mask on GpSimd, then
    per head: PV with V_aug stationary (ones column -> denominator row),
    denominator reshaped [1,1024]->[128,8] through DRAM so the plain
    DVE reciprocal runs partition-parallel, [64,1024] broadcast DMA'd
    straight from DRAM with a stride-0 read, DVE normalize.
"""

import os
import numpy as np
from contextlib import ExitStack

import concourse.bass as bass
import concourse.mybir as mybir
import concourse.tile as tile
from concourse.bass import ds, ts, AP
from concourse.bass_utils import run_bass_kernel_spmd

F32 = mybir.dt.float32
BF16 = mybir.dt.bfloat16

B, T, C, H = 16, 1024, 768, 12
D = C // H           # 64
NCORES = 8
B_LOC = B // NCORES  # 2
KT = C // 128        # 6 contraction tiles
TT = T // 128        # 8 token tiles
NPAIR = H // 2       # 6 head pairs
EXP = mybir.ActivationFunctionType.Exp


def split_multi_waits(nc):
    """Hoist surplus sync waits onto standalone EventSemaphore instructions.

    The walrus build in this environment rejects any instruction carrying
    more than one sync wait ("Too many sync wait commands"). Engine queues
    execute in order, so waiting on each semaphore in a preceding
    EventSemaphore instruction is equivalent to waiting on all of them at
    the original instruction.
    """
    n_split = 0
    for f in nc.m.functions:
        for blk in f.blocks:
            out = []
            for inst in blk.instructions:
                si = inst.sync_info
                if si is not None and si.on_wait and len(si.on_wait) > 1:
                    waits = list(si.on_wait)
                    for w in waits[:-1]:
                        n_split += 1
                        ev = mybir.InstEventSemaphore(
                            name=f"I-waitsplit-{n_split}",
                            ins=[],
                            outs=[],
                            engine=inst.engine,
                            sync_info=mybir.SyncInfo(on_wait=[w], on_update=[]),
                        )
                        out.append(ev)
                    si.on_wait = waits[-1:]
                out.append(inst)
            blk.instructions[:] = out
    return n_split


def build_program(split_waits=True, level=None):
    if level is None:
        level = int(os.environ.get("BUILD_LEVEL", "5"))
    nc = bass.Bass()
    x = nc.declare_dram_parameter("x", [B_LOC, T, C], BF16, isOutput=False)
    wqkv = nc.declare_dram_parameter("wqkv", [C, 3 * C], BF16, isOutput=False)
    wproj = nc.declare_dram_parameter("wproj", [C, C], BF16, isOutput=False)
    bqkt = nc.declare_dram_parameter("bqkt", [128, 2 * NPAIR], F32, isOutput=False)
    bvbc = nc.declare_dram_parameter("bvbc", [128, C], F32, isOutput=False)
    bobc = nc.declare_dram_parameter("bobc", [128, C], F32, isOutput=False)
    maskut = nc.declare_dram_parameter("maskut", [128, 128], BF16, isOutput=False)
    onesb = nc.declare_dram_parameter("onesb", [128, D], BF16, isOutput=False)
    out = nc.declare_dram_parameter("out", [B_LOC, T, C], F32, isOutput=True)

    with tile.TileContext(nc) as tc, ExitStack() as ctx, \
            nc.allow_low_precision(reason="bf16 matmul operands"):
        consts = ctx.enter_context(tc.tile_pool(name="consts", bufs=1))
        wq_pool = ctx.enter_context(tc.tile_pool(name="wq", bufs=1))
        wp_pool = ctx.enter_context(tc.tile_pool(name="wp", bufs=1))
        xt_pool = ctx.enter_context(tc.tile_pool(name="xt", bufs=1))
        qk_pool = ctx.enter_context(tc.tile_pool(name="qk", bufs=4))
        va_pool = ctx.enter_context(tc.tile_pool(name="va", bufs=1))
        pexp = ctx.enter_context(tc.tile_pool(name="pexp", bufs=17))
        yt_pool = ctx.enter_context(tc.tile_pool(name="yt", bufs=1))
        lpool = ctx.enter_context(tc.tile_pool(name="lpool", bufs=2))
        ostage = ctx.enter_context(tc.tile_pool(name="ostage", bufs=2))
        st_pool = ctx.enter_context(tc.tile_pool(name="st", bufs=2, space="PSUM"))
        py_pool = ctx.enter_context(tc.tile_pool(name="py", bufs=1, space="PSUM"))
        pq_pool = ctx.enter_context(tc.tile_pool(name="pq", bufs=2, space="PSUM"))
        dram = ctx.enter_context(tc.tile_pool(name="dram", bufs=1, space="DRAM"))
        scr1 = dram.tile([2 * H, T], F32, tag="scr1", name="scr1")
        scr2 = dram.tile([2 * H, T], F32, tag="scr2", name="scr2")

        # ---- prologue DMAs.  SP queue: x^T(b0) per-k transposes interleaved
        # with the matching W_qkv k-tiles (the first QK job streams k-by-k).
        # ACT queue: consts, x^T(b1), W_proj.
        xt_tiles = {}
        wq = [None] * KT

        mask_sb = consts.tile([128, 128], BF16)
        nc.scalar.dma_start(mask_sb[:], maskut[:])
        bqk_sb = consts.tile([128, 2 * NPAIR], F32)
        nc.scalar.dma_start(bqk_sb[:], bqkt[:])

        for k in range(KT):
            xt = xt_pool.tile([128, T], BF16, tag=f"xt0{k}", name=f"xt0{k}")
            nc.sync.dma_start_transpose(
                xt[:].rearrange("p (o t) -> p o t", o=1), x[0, :, ts(k, 128)]
            )
            xt_tiles[(0, k)] = xt
            wt = wq_pool.tile([128, 3 * C], BF16, tag=f"wq{k}", name=f"wq{k}")
            nc.sync.dma_start(wt[:], wqkv[ts(k, 128), :])
            wq[k] = wt

        for k in range(KT):
            xt = xt_pool.tile([128, T], BF16, tag=f"xt1{k}", name=f"xt1{k}")
            nc.sync.dma_start_transpose(
                xt[:].rearrange("p (o t) -> p o t", o=1), x[1, :, ts(k, 128)]
            )
            xt_tiles[(1, k)] = xt

        bvbc_sb = consts.tile([128, C], F32)
        nc.scalar.dma_start(bvbc_sb[:], bvbc[:])
        bobc_sb = consts.tile([128, C], F32)
        nc.scalar.dma_start(bobc_sb[:], bobc[:])
        ones_sb = consts.tile([128, D], BF16)
        nc.scalar.dma_start(ones_sb[:], onesb[:])
        wp = []
        for k in range(KT):
            wt = wp_pool.tile([128, C], BF16, tag=f"wp{k}", name=f"wp{k}")
            nc.scalar.dma_start(wt[:], wproj[ts(k, 128), :])
            wp.append(wt)

        def xts(b, k):
            return xt_tiles[(b, k)]

        qk_tiles = {}   # (b, p, 'q'|'k') -> tile
        va_tiles = {}   # (b, tt) -> va tile
        yt_tiles = {}   # (b, k) -> yT tile

        # ---------- filler jobs (one PSUM bank each) ----------
        def qk_job(p, b, dst):
            colbase = (0 if dst == "q" else C) + 128 * p
            bcol = p if dst == "q" else NPAIR + p
            dtile = qk_pool.tile([128, T], BF16, tag=dst, name=f"{dst}{b}p{p}")
            qk_tiles[(b, p, dst)] = dtile
            for half in range(2):
                pq = pq_pool.tile([128, 512], F32, tag="pq", name="pq")
                for k in range(KT):
                    nc.tensor.matmul(
                        pq[:],
                        lhsT=wq[k][:, ds(colbase, 128)],
                        rhs=xts(b, k)[:, ds(512 * half, 512)],
                        start=(k == 0),
                        stop=(k == KT - 1),
                    )
                nc.scalar.add(
                    dtile[:, ds(512 * half, 512)],
                    pq[:],
                    bqk_sb[:, ds(bcol, 1)],
                )

        def v_job(b, tt):
            va = va_pool.tile([128, H * (D + 1)], BF16, tag=f"va{b}{tt}",
                              name=f"va{b}{tt}")
            va_tiles[(b, tt)] = va
            va3 = va.rearrange("p (h e) -> p h e", e=D + 1)
            nc.vector.tensor_copy(
                va3[:, :, D:D + 1],
                ones_sb[:, 0:H].rearrange("p (h o) -> p h o", o=1),
            )
            for vo, w, h0, nh in ((0, 512, 0, 8), (512, 256, 8, 4)):
                pv = pq_pool.tile([128, 512], F32, tag="pq", name="pv")
                for k in range(KT):
                    nc.tensor.matmul(
                        pv[:, 0:w],
                        lhsT=xts(b, k)[:, ts(tt, 128)],
                        rhs=wq[k][:, ds(2 * C + vo, w)],
                        start=(k == 0),
                        stop=(k == KT - 1),
                    )
                nc.vector.tensor_add(
                    va3[:, ds(h0, nh), 0:D],
                    pv[:, 0:w].rearrange("p (h e) -> p h e", e=D),
                    bvbc_sb[:, ds(vo, w)].rearrange("p (h e) -> p h e", e=D),
                )

        def op_job(b, tt):
            ot = ostage.tile([128, C], F32, tag="ostage", name="ot")
            for vo, w in ((0, 512), (512, 256)):
                po = pq_pool.tile([128, 512], F32, tag="pq", name="po")
                for k in range(KT):
                    nc.tensor.matmul(
                        po[:, 0:w],
                        lhsT=yt_tiles[(b, k)][:, ts(tt, 128)],
                        rhs=wp[k][:, ds(vo, w)],
                        start=(k == 0),
                        stop=(k == KT - 1),
                    )
                nc.vector.tensor_add(ot[:, ds(vo, w)], po[:, 0:w],
                                     bobc_sb[:, ds(vo, w)])
            nc.sync.dma_start(out[b, ts(tt, 128), :], ot[:])

        # ---------- attention ----------
        def pv_and_norm(b, p, h2, pe_list):
            h = 2 * p + h2
            pb = D * h2
            py = py_pool.tile([D + 1, T], F32, tag="py", name="py")
            for i in range(TT):
                cstart = 128 * i
                chunks = []
                cs = cstart
                while cs < T:
                    w = min(512 - (cs % 512), T - cs)
                    chunks.append((cs, w))
                    cs += w
                for cs, w in chunks[1:] + chunks[:1]:
                    nc.tensor.matmul(
                        py[:, ds(cs, w)],
                        lhsT=va_tiles[(b, i)][:, ds((D + 1) * h, D + 1)],
                        rhs=pe_list[i][:, ds(cs - cstart, w)],
                        start=(i == 0),
                        stop=(i == TT - 1),
                        skip_group_check=True,
                    )
            # normalize: reshape the denominator row through DRAM so the
            # DVE reciprocal runs partition-parallel; broadcast from DRAM.
            hidx = (b * H + h)
            lden = lpool.tile([1, T], F32, tag="lden", name="lden")
            nc.vector.tensor_copy(lden[:], py[ds(D, 1), :])
            yraw = lpool.tile([D, T], BF16, tag="yraw", name="yraw")
            nc.vector.tensor_copy(yraw[:], py[ds(0, D), :])
            nc.sync.dma_start(scr1[hidx, :], lden[0:1, :])
            ldsq = lpool.tile([128, TT], F32, tag="ldsq", name="ldsq")
            nc.sync.dma_start(
                ldsq[:], scr1[hidx, :].rearrange("(p j) -> p j", j=TT)
            )
            lrsq = lpool.tile([128, TT], F32, tag="lrsq", name="lrsq")
            nc.vector.reciprocal(lrsq[:], ldsq[:])
            nc.sync.dma_start(scr2[hidx, :], lrsq[:])
            lbb = lpool.tile([D, T], F32, tag="lbb", name="lbb")
            s1 = scr2[hidx:hidx + 1, :]
            nc.sync.dma_start(
                lbb[:], AP(s1.tensor, s1.offset, [[0, D], [1, T]])
            )
            if (b, p) not in yt_tiles:
                for k in range(KT):
                    yt_tiles[(b, k)] = yt_pool.tile(
                        [128, T], BF16, tag=f"yT{b}{k}", name=f"yT{b}{k}"
                    )
            if pb == 0:
                nc.vector.tensor_mul(
                    yt_tiles[(b, p)][ds(0, D), :], yraw[:], lbb[:]
                )
            else:
                # partition-shifted DVE write into a PE-weight-loaded tile
                # crashes the exec unit; stage at base 0, DMA does the shift.
                ystg = lpool.tile([D, T], BF16, tag="ystg", name="ystg")
                nc.vector.tensor_mul(ystg[:], yraw[:], lbb[:])
                nc.sync.dma_start(yt_tiles[(b, p)][ds(pb, D), :], ystg[:])

        def attention_pair(b, p, fill1, fill2):
            qb = qk_tiles[(b, p, "q")]
            kb = qk_tiles[(b, p, "k")]
            for h2, fill in ((0, fill1), (1, fill2)):
                pe_list = []
                for i in range(TT):
                    cstart = 128 * i
                    wtot = T - cstart
                    st = st_pool.tile([128, T], F32, tag="st", name="st")
                    lc = 0
                    while lc < wtot:
                        w = min(512 - (lc % 512), wtot - lc)
                        nc.tensor.matmul(
                            st[:, ds(lc, w)],
                            lhsT=kb[ds(D * h2, D), ts(i, 128)],
                            rhs=qb[ds(D * h2, D), ds(cstart + lc, w)],
                            start=True,
                            stop=True,
                        )
                        lc += w
                    pe_t = pexp.tile([128, T], BF16, tag="pexp", name="pe_t")
                    nc.scalar.activation(pe_t[:, 0:wtot], st[:, 0:wtot],
                                         EXP, scale=0.125)
                    nc.gpsimd.tensor_mul(pe_t[:, 0:128], pe_t[:, 0:128],
                                         mask_sb[:])
                    pe_list.append(pe_t)
                for job in fill:
                    job()
                pv_and_norm(b, p, h2, pe_list)

        # ---------- filler schedule ----------
        def qk_pair_jobs(p, b):
            return [lambda p=p, b=b: qk_job(p, b, "q"),
                    lambda p=p, b=b: qk_job(p, b, "k")]

        slots = []  # (b, p, fill1, fill2)
        slots.append((0, 0, qk_pair_jobs(1, 0),
                      [lambda: v_job(1, 0), lambda: v_job(1, 1)]))
        slots.append((0, 1, qk_pair_jobs(2, 0),
                      [lambda: v_job(1, 2), lambda: v_job(1, 3)]))
        slots.append((0, 2, qk_pair_jobs(3, 0),
                      [lambda: v_job(1, 4), lambda: v_job(1, 5)]))
        slots.append((0, 3, qk_pair_jobs(4, 0),
                      [lambda: v_job(1, 6), lambda: v_job(1, 7)]))
        slots.append((0, 4, qk_pair_jobs(5, 0), qk_pair_jobs(0, 1)))
        slots.append((0, 5, qk_pair_jobs(1, 1), []))
        slots.append((1, 0, qk_pair_jobs(2, 1),
                      [lambda: op_job(0, 0), lambda: op_job(0, 1)]))
        slots.append((1, 1, qk_pair_jobs(3, 1),
                      [lambda: op_job(0, 2), lambda: op_job(0, 3)]))
        slots.append((1, 2, qk_pair_jobs(4, 1),
                      [lambda: op_job(0, 4), lambda: op_job(0, 5)]))
        slots.append((1, 3, qk_pair_jobs(5, 1),
                      [lambda: op_job(0, 6), lambda: op_job(0, 7)]))
        slots.append((1, 4, [], []))
        slots.append((1, 5, [], []))

        # ---------- prologue compute ----------
        qk_job(0, 0, "q")
        qk_job(0, 0, "k")
        for tt in range(TT):
            v_job(0, tt)

        for b, p, fill1, fill2 in slots:
            attention_pair(b, p, fill1, fill2)

        # ---------- tail: out-proj of batch 1 ----------
        for tt in range(TT):
            op_job(1, tt)

    if split_waits:
        split_multi_waits(nc)
    return nc


def make_in_maps(x, W_qkv, b_qkv, W_proj, b_proj):
    import ml_dtypes

    bf16 = ml_dtypes.bfloat16
    x = np.ascontiguousarray(np.asarray(x, dtype=np.float32)).astype(bf16)
    W_qkv = np.ascontiguousarray(np.asarray(W_qkv, dtype=np.float32)).astype(bf16)
    W_proj = np.ascontiguousarray(np.asarray(W_proj, dtype=np.float32)).astype(bf16)
    b_qkv = np.asarray(b_qkv, dtype=np.float32)
    b_proj = np.asarray(b_proj, dtype=np.float32)

    bqkt = np.ascontiguousarray(b_qkv[: 2 * C].reshape(2 * NPAIR, 128).T)
    bvbc = np.ascontiguousarray(np.tile(b_qkv[2 * C:].reshape(1, C), (128, 1)))
    bobc = np.ascontiguousarray(np.tile(b_proj.reshape(1, C), (128, 1)))
    maskut = np.triu(np.ones((128, 128), dtype=np.float32)).astype(bf16)
    onesb = np.ones((128, D), dtype=np.float32).astype(bf16)

    shared = {
        "wqkv": W_qkv,
        "wproj": W_proj,
        "bqkt": bqkt,
        "bvbc": bvbc,
        "bobc": bobc,
        "maskut": maskut,
        "onesb": onesb,
    }
    in_maps = []
    for c in range(NCORES):
        m = dict(shared)
        m["x"] = np.ascontiguousarray(x[B_LOC * c: B_LOC * (c + 1)])
        in_maps.append(m)
    return in_maps


_PROGRAM = None


def kernel(x, W_qkv, b_qkv, W_proj, b_proj):
    global _PROGRAM
    if _PROGRAM is None:
        _PROGRAM = build_program()
    in_maps = make_in_maps(x, W_qkv, b_qkv, W_proj, b_proj)
    res = run_bass_kernel_spmd(_PROGRAM, in_maps, list(range(NCORES)))
    out = np.concatenate([res.results[c]["out"] for c in range(NCORES)], axis=0)
    return out.astype(np.float32)


if __name__ == "__main__":
    nc = build_program()
    print("built ok; instructions:",
          sum(len(bb.instructions) for f in nc.m.functions for bb in f.blocks))
